# revision 7
# baseline (speedup 1.0000x reference)
"""v3: transposed-layout NeRFLoss kernel — scan-free distortion via PE matmuls.

Distortion identity (per ray, inclusive cumsums, m4 = 4m, iv2 = 2*intervals):
    4*loss = sum_i sq_i * (2/3)*(Hu z)_i  +  sum_i w_i * (D (w*m4))_i
where (Hu z)_i = iv2_i (banded stencil with clamped edges),
      (Tm z)_i = m4_i  (banded stencil with clamped edges),
      D[i,j]  = sign(j-i)   [D(wm4) = rev_cumsum - cumsum, which absorbs
                             W*WM4 + sum sq*m4 - 2*sum w*cwm4 exactly].
All three are matrices applied along the sample axis -> PE matmuls in a
transposed layout ([samples on partitions, rays on free]).  No scan, no
per-ray boundary terms.  Samples padded 192->256 (zeros), bf16.
"""

import numpy as np
import ml_dtypes

import concourse.bass as bass
import concourse.mybir as mybir
from concourse.tile import TileContext
from concourse.bass_utils import run_bass_kernel_spmd

N_RAYS, S, C = 65536, 192, 7
N_CORES = 8
N_LOC = N_RAYS // N_CORES        # 8192 rays per core
S2 = 256                         # padded samples (2 chunks of 128)
RT = 512                         # rays per tile
NT = N_LOC // RT                 # 16 tiles per core
GC = N_LOC // 128                # 64 rays per partition (grouped layout)
LAMBDA_DIST = 1e-4

F32 = mybir.dt.float32
BF16 = mybir.dt.bfloat16
Alu = mybir.AluOpType
Act = mybir.ActivationFunctionType
AX = mybir.AxisListType
BF = ml_dtypes.bfloat16

# parts columns
C_H, C_D = 0, NT                 # sH cols 0:16, sD cols 16:32
C_LSE, C_PICK = 2 * NT, 2 * NT + 1
P_COLS = 2 * NT + 2

_CACHE = {}


def _legalize_waits(nc, max_waits=1):
    k = 0
    for fn in nc.m.functions:
        for bb in fn.blocks:
            out = []
            for ins in bb.instructions:
                si = getattr(ins, "sync_info", None)
                waits = list(si.on_wait) if si is not None and si.on_wait else []
                if len(waits) > max_waits:
                    for w in waits[:-max_waits]:
                        k += 1
                        out.append(mybir.InstNoOp(
                            name=f"waitsplit-{k}", ins=[], outs=[],
                            engine=ins.engine,
                            sync_info=mybir.SyncInfo(on_wait=[w], on_update=[]),
                        ))
                    ins.sync_info = mybir.SyncInfo(
                        on_wait=waits[-max_waits:], on_update=si.on_update)
                out.append(ins)
            if len(out) != len(bb.instructions):
                bb.instructions[:] = out
    return k


OV = 126                        # chunk1' starts at sample 126 (overlap 2)
SPLIT = 127                     # ownership split between chunks
_SMAP = [np.arange(128), OV + np.arange(128)]


def _matrices():
    Tm = np.zeros((S2, S2))
    Hu = np.zeros((S2, S2))
    for i in range(1, S - 1):
        Tm[i, i - 1], Tm[i, i], Tm[i, i + 1] = 1, 2, 1
        Hu[i, i - 1], Hu[i, i + 1] = -1, 1
    Tm[0, 0], Tm[0, 1] = 3, 1
    Tm[S - 1, S - 2], Tm[S - 1, S - 1] = 1, 3
    Hu[0, 0], Hu[0, 1] = -1, 1
    Hu[S - 1, S - 2], Hu[S - 1, S - 1] = -1, 1
    idx = np.arange(S2)
    D = np.sign(np.subtract.outer(idx, idx)) * -1.0  # D[i,j] = sign(j-i)
    return Tm, Hu, D


def _lhsT_blocks():
    """Overlapping-chunk lhsT blocks.  Chunk c covers samples _SMAP[c];
    banded matrices (Tm, Hu) need only the same-chunk contraction (rows
    owned per SPLIT), the dense D needs all four (k, c) with ownership
    masks so overlapped samples count exactly once.
    Block order: TmB[0], TmB[1], HuB[0], HuB[1], D(0,0), D(0,1), D(1,0),
    D(1,1) — one [128, 8*128] tensor."""
    Tm, Hu, D = _matrices()
    cols = []
    for M in (Tm, Hu):
        for c in range(2):
            B = np.zeros((128, 128))
            for ip in range(128):
                i = _SMAP[c][ip]
                ok = (i < SPLIT) if c == 0 else (SPLIT <= i < S)
                if not ok:
                    continue
                for jp in range(128):
                    B[jp, ip] = M[i, _SMAP[c][jp]]
            cols.append(B)
    for k in range(2):
        for c in range(2):
            B = np.zeros((128, 128))
            for jp in range(128):
                j = _SMAP[k][jp]
                ok_in = (j < SPLIT) if k == 0 else (SPLIT <= j < S)
                if not ok_in:
                    continue
                for ip in range(128):
                    i = _SMAP[c][ip]
                    ok_out = (i < 128) if c == 0 else (128 <= i < S)
                    if not ok_out:
                        continue
                    B[jp, ip] = D[i, j]
            cols.append(B)
    return np.concatenate(cols, axis=1).astype(BF)


def build_nc(legalize=True):
    nc = bass.Bass("TRN2", target_bir_lowering=False)

    wT_d = nc.dram_tensor("wT", [S2, N_LOC], BF16, kind="ExternalInput")
    zT_d = nc.dram_tensor("zT", [S2, N_LOC], BF16, kind="ExternalInput")
    sem_d = nc.dram_tensor("sem", [128, GC * C], F32, kind="ExternalInput")
    oh_d = nc.dram_tensor("onehot", [128, GC * C], F32, kind="ExternalInput")
    dep_d = nc.dram_tensor("dep", [128, GC], F32, kind="ExternalInput")
    mk4_d = nc.dram_tensor("mask4", [128, GC], F32, kind="ExternalInput")
    rgb_d = nc.dram_tensor("rgb", [128, 192], F32, kind="ExternalInput")
    tgt_d = nc.dram_tensor("tgt", [128, 192], F32, kind="ExternalInput")

    rgbo_d = nc.dram_tensor("rgb_out", [128, 192], F32, kind="ExternalOutput")
    sky_d = nc.dram_tensor("sky_out", [128, GC], F32, kind="ExternalOutput")
    parts_d = nc.dram_tensor("parts", [128, P_COLS], F32, kind="ExternalOutput")

    blocks_d = nc.inline_tensor(_lhsT_blocks(), name="lhsT_blocks")

    v = nc.vector
    sc = nc.scalar

    with TileContext(nc) as tc:
        with tc.tile_pool(name="const", bufs=1) as constp, \
             tc.tile_pool(name="io", bufs=6) as io, \
             tc.tile_pool(name="work", bufs=4) as work, \
             tc.tile_pool(name="small", bufs=2) as small, \
             tc.tile_pool(name="acc", bufs=1) as accp, \
             tc.tile_pool(name="psA", bufs=1, space="PSUM") as psA, \
             tc.tile_pool(name="psB", bufs=1, space="PSUM") as psB, \
             tc.tile_pool(name="psC", bufs=2, space="PSUM") as psC:

            blk = constp.tile([128, 8 * 128], BF16)
            nc.gpsimd.dma_start(blk[:], blocks_d[:])

            def lb(b):
                return blk[:, 128 * b:128 * (b + 1)]

            parts = accp.tile([128, P_COLS], F32)

            # ---------------- small per-ray losses (fp32) ----------------
            rgbt = small.tile([128, 192], F32, tag="rgbt")
            tgtt = small.tile([128, 192], F32, tag="tgtt")
            nc.gpsimd.dma_start(rgbt[:], rgb_d[:])
            nc.gpsimd.dma_start(tgtt[:], tgt_d[:])
            df = small.tile([128, 192], F32, tag="df")
            v.tensor_sub(df[:], rgbt[:], tgtt[:])
            sc.activation(df[:], df[:], Act.Square)
            nc.gpsimd.dma_start(rgbo_d[:], df[:])

            dept = small.tile([128, GC], F32, tag="dept")
            mk4t = small.tile([128, GC], F32, tag="mk4t")
            nc.gpsimd.dma_start(dept[:], dep_d[:])
            nc.gpsimd.dma_start(mk4t[:], mk4_d[:])
            e4 = small.tile([128, GC], F32, tag="e4")
            sc.activation(e4[:], dept[:], Act.Exp, scale=-1.0)
            skyt = small.tile([128, GC], F32, tag="skyt")
            v.scalar_tensor_tensor(skyt[:], mk4t[:], 0.1, e4[:], Alu.mult,
                                   Alu.mult)
            nc.gpsimd.dma_start(sky_d[:], skyt[:])

            semt = small.tile([128, GC * C], F32, tag="semt")
            oht = small.tile([128, GC * C], F32, tag="oht")
            nc.gpsimd.dma_start(semt[:], sem_d[:])
            nc.gpsimd.dma_start(oht[:], oh_d[:])
            expt = small.tile([128, GC * C], F32, tag="expt")
            sc.activation(expt[:], semt[:], Act.Exp)
            ssum = small.tile([128, GC], F32, tag="ssum")
            v.reduce_sum(ssum[:], expt[:].rearrange("p (g c) -> p g c", c=C),
                         axis=AX.X)
            lses = small.tile([128, GC], F32, tag="lses")
            sc.activation(lses[:], ssum[:], Act.Ln,
                          accum_out=parts[:, C_LSE:C_LSE + 1])
            pick = small.tile([128, GC * C], F32, tag="pick")
            v.scalar_tensor_tensor(pick[:], oht[:], 1.0, semt[:],
                                   Alu.mult, Alu.mult,
                                   accum_out=parts[:, C_PICK:C_PICK + 1])

            # ---------------- distortion (transposed, bf16+PE) ----------
            for t in range(NT):
                r0 = RT * t
                wT = io.tile([128, 2 * RT], BF16, tag="wT")
                zT = io.tile([128, 2 * RT], BF16, tag="zT")
                nc.sync.dma_start(wT[:, 0:RT], wT_d[0:128, r0:r0 + RT])
                nc.sync.dma_start(wT[:, RT:2 * RT],
                                  wT_d[OV:OV + 128, r0:r0 + RT])
                nc.scalar.dma_start(zT[:, 0:RT], zT_d[0:128, r0:r0 + RT])
                nc.scalar.dma_start(zT[:, RT:2 * RT],
                                    zT_d[OV:OV + 128, r0:r0 + RT])

                m4T = psA.tile([128, 2 * RT], F32, tag="m4T")
                huz = psB.tile([128, 2 * RT], F32, tag="huz")
                for c in range(2):
                    nc.tensor.matmul(m4T[:, c * RT:(c + 1) * RT],
                                     lb(0 + c), zT[:, c * RT:(c + 1) * RT],
                                     start=True, stop=True)
                for c in range(2):
                    nc.tensor.matmul(huz[:, c * RT:(c + 1) * RT],
                                     lb(2 + c), zT[:, c * RT:(c + 1) * RT],
                                     start=True, stop=True)

                m4b = work.tile([128, 2 * RT], BF16, tag="m4b")
                sc.copy(m4b[:], m4T[:])
                wm4T = work.tile([128, 2 * RT], BF16, tag="wm4T")
                v.tensor_mul(wm4T[:], wT[:], m4b[:])

                dwT = psC.tile([128, 2 * RT], F32, tag="dwT")
                for c in range(2):
                    for k in range(2):
                        nc.tensor.matmul(dwT[:, c * RT:(c + 1) * RT],
                                         lb(4 + 2 * k + c),
                                         wm4T[:, k * RT:(k + 1) * RT],
                                         start=(k == 0), stop=(k == 1))

                sqT = work.tile([128, 2 * RT], BF16, tag="sqT")
                sc.activation(sqT[:], wT[:], Act.Square)

                scr = work.tile([128, 2 * RT], BF16, tag="scr")
                v.scalar_tensor_tensor(scr[:], sqT[:], 2.0 / 3.0, huz[:],
                                       Alu.mult, Alu.mult,
                                       accum_out=parts[:, C_H + t:C_H + t + 1])
                scr2 = work.tile([128, 2 * RT], BF16, tag="scr2")
                v.scalar_tensor_tensor(
                    scr2[:], wT[:], 1.0, dwT[:], Alu.mult, Alu.mult,
                    accum_out=parts[:, C_D + t:C_D + t + 1])

            nc.gpsimd.dma_start(parts_d[:], parts[:])

    if legalize:
        _legalize_waits(nc)
    nc.finalize()
    return nc


def _pack_core(core, rgb0, wTp, zTp, semantic0, depth0, target_rgb,
               onehot, mask4):
    lo, hi = core * N_LOC, (core + 1) * N_LOC
    return {
        "wT": np.ascontiguousarray(wTp[:, lo:hi]),
        "zT": np.ascontiguousarray(zTp[:, lo:hi]),
        "sem": np.ascontiguousarray(semantic0[lo:hi]).reshape(128, GC * C),
        "onehot": np.ascontiguousarray(onehot[lo:hi]).reshape(128, GC * C),
        "dep": np.ascontiguousarray(depth0[lo:hi]).reshape(128, GC),
        "mask4": np.ascontiguousarray(mask4[lo:hi]).reshape(128, GC),
        "rgb": np.ascontiguousarray(rgb0[lo:hi]).reshape(128, 192),
        "tgt": np.ascontiguousarray(target_rgb[lo:hi]).reshape(128, 192),
    }


def host_inputs(rgb0, opacity0, ws0, z_vals0, semantic0, depth0, target_rgb,
                label, stages):
    rgb0 = np.asarray(rgb0, dtype=np.float32)
    semantic0 = np.asarray(semantic0, dtype=np.float32)
    depth0 = np.asarray(depth0, dtype=np.float32)
    target_rgb = np.asarray(target_rgb, dtype=np.float32)
    label = np.asarray(label)
    onehot = (label[:, None] == np.arange(C)[None, :]).astype(np.float32)
    mask4 = (label == 4).astype(np.float32)

    wTp = np.zeros((S2, N_RAYS), dtype=BF)
    zTp = np.zeros((S2, N_RAYS), dtype=BF)
    wTp[:S] = np.asarray(ws0, dtype=np.float32).astype(BF).T
    zTp[:S] = np.asarray(z_vals0, dtype=np.float32).astype(BF).T
    return [
        _pack_core(c, rgb0, wTp, zTp, semantic0, depth0, target_rgb,
                   onehot, mask4)
        for c in range(N_CORES)
    ]


def assemble(results):
    rgb_loss = np.concatenate(
        [r["rgb_out"].reshape(N_LOC, 3) for r in results], axis=0)
    sky = np.concatenate(
        [r["sky_out"].reshape(N_LOC) for r in results], axis=0)

    dist_sum = 0.0
    lse_sum = 0.0
    pick_sum = 0.0
    for r in results:
        p = r["parts"].astype(np.float64)
        dist_sum += 0.25 * (p[:, C_H:C_H + NT].sum()
                            + p[:, C_D:C_D + NT].sum())
        lse_sum += p[:, C_LSE].sum()
        pick_sum += p[:, C_PICK].sum()

    dist = np.float32(LAMBDA_DIST * dist_sum / N_RAYS)
    ce = np.float32(0.04 * (lse_sum - pick_sum) / N_RAYS)
    return rgb_loss, dist, ce, sky


def kernel(rgb0, opacity0, ws0, z_vals0, semantic0, depth0, target_rgb,
           label, stages, _trace=False):
    if "nc" not in _CACHE:
        _CACHE["nc"] = build_nc()
    nc = _CACHE["nc"]
    in_maps = host_inputs(rgb0, opacity0, ws0, z_vals0, semantic0, depth0,
                          target_rgb, label, stages)
    res = run_bass_kernel_spmd(nc, in_maps, core_ids=list(range(N_CORES)),
                               trace=_trace)
    out = assemble(res.results)
    if _trace:
        return out, res
    return out


# revision 9
# speedup vs baseline: 1.0798x; 1.0798x over previous
"""v3: transposed-layout NeRFLoss kernel — scan-free distortion via PE matmuls.

Distortion identity (per ray, inclusive cumsums, m4 = 4m, iv2 = 2*intervals):
    4*loss = sum_i sq_i * (2/3)*(Hu z)_i  +  sum_i w_i * (D (w*m4))_i
where (Hu z)_i = iv2_i (banded stencil with clamped edges),
      (Tm z)_i = m4_i  (banded stencil with clamped edges),
      D[i,j]  = sign(j-i)   [D(wm4) = rev_cumsum - cumsum, which absorbs
                             W*WM4 + sum sq*m4 - 2*sum w*cwm4 exactly].
All three are matrices applied along the sample axis -> PE matmuls in a
transposed layout ([samples on partitions, rays on free]).  No scan, no
per-ray boundary terms.  Samples padded 192->256 (zeros), bf16.
"""

import numpy as np
import ml_dtypes

import concourse.bass as bass
import concourse.mybir as mybir
from concourse.tile import TileContext
from concourse.bass_utils import run_bass_kernel_spmd

N_RAYS, S, C = 65536, 192, 7
N_CORES = 8
N_LOC = N_RAYS // N_CORES        # 8192 rays per core
S2 = 256                         # padded samples (2 chunks of 128)
RT = 512                         # rays per tile
NT = N_LOC // RT                 # 16 tiles per core
GC = N_LOC // 128                # 64 rays per partition (grouped layout)
LAMBDA_DIST = 1e-4

F32 = mybir.dt.float32
BF16 = mybir.dt.bfloat16
Alu = mybir.AluOpType
Act = mybir.ActivationFunctionType
AX = mybir.AxisListType
BF = ml_dtypes.bfloat16

# parts columns
C_H, C_D = 0, NT                 # sH cols 0:16, sD cols 16:32
C_LSE, C_PICK = 2 * NT, 2 * NT + 1
P_COLS = 2 * NT + 2

_CACHE = {}


def _legalize_waits(nc, max_waits=1):
    k = 0
    for fn in nc.m.functions:
        for bb in fn.blocks:
            out = []
            for ins in bb.instructions:
                si = getattr(ins, "sync_info", None)
                waits = list(si.on_wait) if si is not None and si.on_wait else []
                if len(waits) > max_waits:
                    for w in waits[:-max_waits]:
                        k += 1
                        out.append(mybir.InstNoOp(
                            name=f"waitsplit-{k}", ins=[], outs=[],
                            engine=ins.engine,
                            sync_info=mybir.SyncInfo(on_wait=[w], on_update=[]),
                        ))
                    ins.sync_info = mybir.SyncInfo(
                        on_wait=waits[-max_waits:], on_update=si.on_update)
                out.append(ins)
            if len(out) != len(bb.instructions):
                bb.instructions[:] = out
    return k


OV = 126                        # chunk1' starts at sample 126 (overlap 2)
SPLIT = 127                     # ownership split between chunks
_SMAP = [np.arange(128), OV + np.arange(128)]


def _matrices():
    Tm = np.zeros((S2, S2))
    Hu = np.zeros((S2, S2))
    for i in range(1, S - 1):
        Tm[i, i - 1], Tm[i, i], Tm[i, i + 1] = 1, 2, 1
        Hu[i, i - 1], Hu[i, i + 1] = -1, 1
    Tm[0, 0], Tm[0, 1] = 3, 1
    Tm[S - 1, S - 2], Tm[S - 1, S - 1] = 1, 3
    Hu[0, 0], Hu[0, 1] = -1, 1
    Hu[S - 1, S - 2], Hu[S - 1, S - 1] = -1, 1
    idx = np.arange(S2)
    D = np.sign(np.subtract.outer(idx, idx)) * -1.0  # D[i,j] = sign(j-i)
    return Tm, Hu, D


def _lhsT_blocks():
    """Overlapping-chunk lhsT blocks.  Chunk c covers samples _SMAP[c];
    banded matrices (Tm, Hu) need only the same-chunk contraction (rows
    owned per SPLIT), the dense D needs all four (k, c) with ownership
    masks so overlapped samples count exactly once.
    Block order: TmB[0], TmB[1], HuB[0], HuB[1], D(0,0), D(0,1), D(1,0),
    D(1,1) — one [128, 8*128] tensor."""
    Tm, Hu, D = _matrices()
    cols = []
    for M in (Tm, Hu):
        for c in range(2):
            B = np.zeros((128, 128))
            for ip in range(128):
                i = _SMAP[c][ip]
                ok = (i < SPLIT) if c == 0 else (SPLIT <= i < S)
                if not ok:
                    continue
                for jp in range(128):
                    B[jp, ip] = M[i, _SMAP[c][jp]]
            cols.append(B)
    for k in range(2):
        for c in range(2):
            B = np.zeros((128, 128))
            for jp in range(128):
                j = _SMAP[k][jp]
                ok_in = (j < SPLIT) if k == 0 else (SPLIT <= j < S)
                if not ok_in:
                    continue
                for ip in range(128):
                    i = _SMAP[c][ip]
                    ok_out = (i < 128) if c == 0 else (128 <= i < S)
                    if not ok_out:
                        continue
                    B[jp, ip] = D[i, j]
            cols.append(B)
    return np.concatenate(cols, axis=1).astype(BF)


def build_nc(legalize=True):
    nc = bass.Bass("TRN2", target_bir_lowering=False)

    wT_d = nc.dram_tensor("wT", [S2, N_LOC], BF16, kind="ExternalInput")
    zT_d = nc.dram_tensor("zT", [S2, N_LOC], BF16, kind="ExternalInput")
    sem_d = nc.dram_tensor("sem", [128, GC * C], F32, kind="ExternalInput")
    oh_d = nc.dram_tensor("onehot", [128, GC * C], F32, kind="ExternalInput")
    dep_d = nc.dram_tensor("dep", [128, GC], F32, kind="ExternalInput")
    mk4_d = nc.dram_tensor("mask4", [128, GC], F32, kind="ExternalInput")
    rgb_d = nc.dram_tensor("rgb", [128, 192], F32, kind="ExternalInput")
    tgt_d = nc.dram_tensor("tgt", [128, 192], F32, kind="ExternalInput")

    rgbo_d = nc.dram_tensor("rgb_out", [128, 192], F32, kind="ExternalOutput")
    sky_d = nc.dram_tensor("sky_out", [128, GC], F32, kind="ExternalOutput")
    parts_d = nc.dram_tensor("parts", [128, P_COLS], F32, kind="ExternalOutput")

    blocks_d = nc.inline_tensor(_lhsT_blocks(), name="lhsT_blocks")

    v = nc.vector
    sc = nc.scalar

    with TileContext(nc) as tc:
        with tc.tile_pool(name="const", bufs=1) as constp, \
             tc.tile_pool(name="io", bufs=4) as io, \
             tc.tile_pool(name="work", bufs=3) as work, \
             tc.tile_pool(name="small", bufs=2) as small, \
             tc.tile_pool(name="acc", bufs=1) as accp, \
             tc.tile_pool(name="psA", bufs=1, space="PSUM") as psA, \
             tc.tile_pool(name="psB", bufs=1, space="PSUM") as psB, \
             tc.tile_pool(name="psC", bufs=2, space="PSUM") as psC:

            blk = constp.tile([128, 8 * 128], BF16)
            nc.gpsimd.dma_start(blk[:], blocks_d[:])

            def lb(b):
                return blk[:, 128 * b:128 * (b + 1)]

            parts = accp.tile([128, P_COLS], F32)

            # ---------------- small per-ray losses (fp32) ----------------
            rgbt = small.tile([128, 192], F32, tag="rgbt")
            tgtt = small.tile([128, 192], F32, tag="tgtt")
            nc.gpsimd.dma_start(rgbt[:], rgb_d[:])
            nc.gpsimd.dma_start(tgtt[:], tgt_d[:])
            df = small.tile([128, 192], F32, tag="df")
            v.tensor_sub(df[:], rgbt[:], tgtt[:])
            sc.activation(df[:], df[:], Act.Square)
            nc.gpsimd.dma_start(rgbo_d[:], df[:])

            dept = small.tile([128, GC], F32, tag="dept")
            mk4t = small.tile([128, GC], F32, tag="mk4t")
            nc.gpsimd.dma_start(dept[:], dep_d[:])
            nc.gpsimd.dma_start(mk4t[:], mk4_d[:])
            e4 = small.tile([128, GC], F32, tag="e4")
            sc.activation(e4[:], dept[:], Act.Exp, scale=-1.0)
            skyt = small.tile([128, GC], F32, tag="skyt")
            v.scalar_tensor_tensor(skyt[:], mk4t[:], 0.1, e4[:], Alu.mult,
                                   Alu.mult)
            nc.gpsimd.dma_start(sky_d[:], skyt[:])

            semt = small.tile([128, GC * C], F32, tag="semt")
            oht = small.tile([128, GC * C], F32, tag="oht")
            nc.gpsimd.dma_start(semt[:], sem_d[:])
            nc.gpsimd.dma_start(oht[:], oh_d[:])
            expt = small.tile([128, GC * C], F32, tag="expt")
            sc.activation(expt[:], semt[:], Act.Exp)
            ssum = small.tile([128, GC], F32, tag="ssum")
            v.reduce_sum(ssum[:], expt[:].rearrange("p (g c) -> p g c", c=C),
                         axis=AX.X)
            lses = small.tile([128, GC], F32, tag="lses")
            sc.activation(lses[:], ssum[:], Act.Ln,
                          accum_out=parts[:, C_LSE:C_LSE + 1])
            pick = small.tile([128, GC * C], F32, tag="pick")
            v.scalar_tensor_tensor(pick[:], oht[:], 1.0, semt[:],
                                   Alu.mult, Alu.mult,
                                   accum_out=parts[:, C_PICK:C_PICK + 1])

            # ---------------- distortion (transposed, bf16+PE) ----------
            for t in range(NT):
                r0 = RT * t
                wT = io.tile([128, 2 * RT], BF16, tag="wT")
                zT = io.tile([128, 2 * RT], BF16, tag="zT")
                nc.sync.dma_start(wT[:, 0:RT], wT_d[0:128, r0:r0 + RT])
                nc.sync.dma_start(wT[:, RT:2 * RT],
                                  wT_d[OV:OV + 128, r0:r0 + RT])
                nc.scalar.dma_start(zT[:, 0:RT], zT_d[0:128, r0:r0 + RT])
                nc.scalar.dma_start(zT[:, RT:2 * RT],
                                    zT_d[OV:OV + 128, r0:r0 + RT])

                m4T = psA.tile([128, 2 * RT], F32, tag="m4T")
                huz = psB.tile([128, 2 * RT], F32, tag="huz")
                for c in range(2):
                    nc.tensor.matmul(m4T[:, c * RT:(c + 1) * RT],
                                     lb(0 + c), zT[:, c * RT:(c + 1) * RT],
                                     start=True, stop=True)
                for c in range(2):
                    nc.tensor.matmul(huz[:, c * RT:(c + 1) * RT],
                                     lb(2 + c), zT[:, c * RT:(c + 1) * RT],
                                     start=True, stop=True)

                m4b = work.tile([128, 2 * RT], BF16, tag="m4b")
                sc.copy(m4b[:], m4T[:])
                wm4T = work.tile([128, 2 * RT], BF16, tag="wm4T")
                v.tensor_mul(wm4T[:], wT[:], m4b[:])

                dwT = psC.tile([128, 2 * RT], F32, tag="dwT")
                for c in range(2):
                    for k in range(2):
                        nc.tensor.matmul(dwT[:, c * RT:(c + 1) * RT],
                                         lb(4 + 2 * k + c),
                                         wm4T[:, k * RT:(k + 1) * RT],
                                         start=(k == 0), stop=(k == 1))

                sqT = work.tile([128, 2 * RT], BF16, tag="sqT")
                sc.activation(sqT[:], wT[:], Act.Square)

                scr = work.tile([128, 2 * RT], BF16, tag="scr")
                v.scalar_tensor_tensor(scr[:], sqT[:], 2.0 / 3.0, huz[:],
                                       Alu.mult, Alu.mult,
                                       accum_out=parts[:, C_H + t:C_H + t + 1])
                scr2 = work.tile([128, 2 * RT], BF16, tag="scr2")
                v.scalar_tensor_tensor(
                    scr2[:], wT[:], 1.0, dwT[:], Alu.mult, Alu.mult,
                    accum_out=parts[:, C_D + t:C_D + t + 1])

            nc.gpsimd.dma_start(parts_d[:], parts[:])

    if legalize:
        _legalize_waits(nc)
    nc.finalize()
    return nc


def _pack_core(core, rgb0, wTp, zTp, semantic0, depth0, target_rgb,
               onehot, mask4):
    lo, hi = core * N_LOC, (core + 1) * N_LOC
    return {
        "wT": np.ascontiguousarray(wTp[:, lo:hi]),
        "zT": np.ascontiguousarray(zTp[:, lo:hi]),
        "sem": np.ascontiguousarray(semantic0[lo:hi]).reshape(128, GC * C),
        "onehot": np.ascontiguousarray(onehot[lo:hi]).reshape(128, GC * C),
        "dep": np.ascontiguousarray(depth0[lo:hi]).reshape(128, GC),
        "mask4": np.ascontiguousarray(mask4[lo:hi]).reshape(128, GC),
        "rgb": np.ascontiguousarray(rgb0[lo:hi]).reshape(128, 192),
        "tgt": np.ascontiguousarray(target_rgb[lo:hi]).reshape(128, 192),
    }


def host_inputs(rgb0, opacity0, ws0, z_vals0, semantic0, depth0, target_rgb,
                label, stages):
    rgb0 = np.asarray(rgb0, dtype=np.float32)
    semantic0 = np.asarray(semantic0, dtype=np.float32)
    depth0 = np.asarray(depth0, dtype=np.float32)
    target_rgb = np.asarray(target_rgb, dtype=np.float32)
    label = np.asarray(label)
    onehot = (label[:, None] == np.arange(C)[None, :]).astype(np.float32)
    mask4 = (label == 4).astype(np.float32)

    wTp = np.zeros((S2, N_RAYS), dtype=BF)
    zTp = np.zeros((S2, N_RAYS), dtype=BF)
    wTp[:S] = np.asarray(ws0, dtype=np.float32).astype(BF).T
    zTp[:S] = np.asarray(z_vals0, dtype=np.float32).astype(BF).T
    return [
        _pack_core(c, rgb0, wTp, zTp, semantic0, depth0, target_rgb,
                   onehot, mask4)
        for c in range(N_CORES)
    ]


def assemble(results):
    rgb_loss = np.concatenate(
        [r["rgb_out"].reshape(N_LOC, 3) for r in results], axis=0)
    sky = np.concatenate(
        [r["sky_out"].reshape(N_LOC) for r in results], axis=0)

    dist_sum = 0.0
    lse_sum = 0.0
    pick_sum = 0.0
    for r in results:
        p = r["parts"].astype(np.float64)
        dist_sum += 0.25 * (p[:, C_H:C_H + NT].sum()
                            + p[:, C_D:C_D + NT].sum())
        lse_sum += p[:, C_LSE].sum()
        pick_sum += p[:, C_PICK].sum()

    dist = np.float32(LAMBDA_DIST * dist_sum / N_RAYS)
    ce = np.float32(0.04 * (lse_sum - pick_sum) / N_RAYS)
    return rgb_loss, dist, ce, sky


def kernel(rgb0, opacity0, ws0, z_vals0, semantic0, depth0, target_rgb,
           label, stages, _trace=False):
    if "nc" not in _CACHE:
        _CACHE["nc"] = build_nc()
    nc = _CACHE["nc"]
    in_maps = host_inputs(rgb0, opacity0, ws0, z_vals0, semantic0, depth0,
                          target_rgb, label, stages)
    res = None
    for attempt in range(3):
        try:
            res = run_bass_kernel_spmd(nc, in_maps,
                                       core_ids=list(range(N_CORES)),
                                       trace=_trace)
            break
        except Exception:
            # first run after a fresh NEFF load occasionally reports
            # NRT_EXEC_UNIT_UNRECOVERABLE; a retry recovers the device
            if attempt == 2:
                raise
            import time
            time.sleep(2.0)
    out = assemble(res.results)
    if _trace:
        return out, res
    return out
